# revision 31
# baseline (speedup 1.0000x reference)
"""Soft decision-tree forward (nn_DTree) on 8 trn2 NeuronCores.

Strategy (pure data parallel, per the sharding hint):
  - shard x row-wise 8 ways; replicate the tiny tree params.
  - host pre-transposes x into the PE lhsT layout (bf16, bias-ones rows baked
    in), so the device does no transposes at all: the SP queue streams xT2
    straight into SBUF.
  - per core: z = [x|1|1] @ [W | -c_hi | -c_lo] via bf16 PE matmuls into fp32
    PSUM, g = sigmoid(z) on ACT (one instruction per 8-tile PSUM wave), then a
    level-by-level value-tree blend
       V_k = g_k * (V_{k+1,L} - V_{k+1,R}) + V_{k+1,R}
    with nodes pre-permuted (level-major, left-children-first).
  - g is stored slot-major [128, slot, 256] so each sigmoid wave is a single
    strided ACT instruction and every level's gates are regular strided slices.
  - the blend is split between DVE (2x bf16 mode) and Pool so the two engines
    finish together: levels 7+6 are blended per sigmoid wave (8 slots, engines
    alternating 5:3), levels 5..0 once per 64-slot group, in place inside the
    v7 buffer. The level-7 delta/beta constants are step-0 broadcast views of
    a tiny [128, 256] upload, so no replicated tiles are ever materialized.
"""

import numpy as np
import ml_dtypes

import concourse.bass as bass
import concourse.bacc as bacc
import concourse.tile as tile
from concourse import mybir
from concourse.bass_utils import run_bass_kernel_spmd

BF16 = ml_dtypes.bfloat16

F = 32
D = 8
NODES = 255
LEAVES = 256
N_FULL = 262144
N_CORES = 8
ROWS = N_FULL // N_CORES  # 32768 rows per core
SLOTS = 64                # tiles per blend group
PW = 8                    # tiles per PSUM/sigmoid wave

# level-major offsets of each level's gates inside the 255-column block
LEVEL_OFF = {7: 0, 6: 128, 5: 192, 4: 224, 3: 240, 2: 248, 1: 252, 0: 254}


def _orderings():
    """ord[k] = local node order at level k (left-children-first recursion)."""
    ordv = {0: [0]}
    for k in range(7):
        ordv[k + 1] = [2 * i for i in ordv[k]] + [2 * i + 1 for i in ordv[k]]
    col_nodes = []
    for k in range(7, -1, -1):
        base = 2 ** k - 1
        col_nodes += [base + i for i in ordv[k]]
    return ordv, np.array(col_nodes)


def host_prep(feature_importances, feature_splits, leaf_node_classes):
    """Tiny-param preprocessing (O(8K) work): relu/sigmoid/c, node permutation,
    bf16 weight matrix with split bias rows, leaf-blend constants."""
    fi = np.asarray(feature_importances, np.float32).reshape(NODES, F)
    fs = np.asarray(feature_splits, np.float32).reshape(NODES, F)
    cls = np.asarray(leaf_node_classes, np.float32).reshape(LEAVES)

    W = np.maximum(fi, 0.0)
    S = 1.0 / (1.0 + np.exp(-fs))
    c = np.sum(W * S, axis=1)  # (NODES,)

    ordv, col_nodes = _orderings()
    Wp = W[col_nodes]          # (255, 32) permuted level-major
    cp = c[col_nodes]

    c_hi = cp.astype(BF16).astype(np.float32)
    c_lo = (cp - c_hi).astype(np.float32)

    wt = np.zeros((128, 256), BF16)
    for b in (0, 64):  # replicate for both class-half partition groups
        wt[b : b + F, 0:NODES] = Wp.T.astype(BF16)
        wt[b + F, 0:NODES] = (-c_hi).astype(BF16)
        wt[b + F + 1, 0:NODES] = (-c_lo).astype(BF16)

    o7 = np.array(ordv[7])
    delta = (cls[2 * o7] - cls[2 * o7 + 1]).astype(BF16)
    beta = cls[2 * o7 + 1].astype(BF16)
    dc = np.zeros((128, 256), BF16)
    dc[:, 0:128] = delta[None, :]
    dc[:, 128:256] = beta[None, :]
    return wt, dc


def host_xt2(x_core):
    """Build the lhsT layout: [128, ROWS//2] bf16.

    Partitions 0-31: features of rows 0..ROWS/2-1 (class 0)
    Partitions 32-33: ones (bias rows)
    Partitions 64-95: features of rows ROWS/2..ROWS-1 (class 1)
    Partitions 96-97: ones
    """
    half = x_core.shape[0] // 2
    xt = np.empty((128, half), BF16)
    xb = np.ascontiguousarray(x_core.astype(BF16).T)  # (32, ROWS) bf16
    xt[0:32] = xb[:, 0:half]
    xt[64:96] = xb[:, half:]
    xt[32:34] = BF16(1.0)
    xt[96:98] = BF16(1.0)
    xt[34:64] = BF16(0.0)
    xt[98:128] = BF16(0.0)
    return xt


def build_nc(rows=ROWS, slots=SLOTS, pw=PW):
    tiles = rows // 128          # 256
    groups = tiles // slots
    half = rows // 2
    assert tiles % slots == 0 and slots % pw == 0
    bf = mybir.dt.bfloat16
    f32 = mybir.dt.float32

    nc = bacc.Bacc()
    xT2_in = nc.dram_tensor("xT2", [128, half], bf, kind="ExternalInput")
    wt_in = nc.dram_tensor("wt", [128, 256], bf, kind="ExternalInput")
    dc_in = nc.dram_tensor("dc", [128, 256], bf, kind="ExternalInput")
    out_dram = nc.dram_tensor("out", [128, tiles], f32, kind="ExternalOutput")

    with tile.TileContext(nc) as tc:
        with (
            tc.tile_pool(name="consts", bufs=1) as consts,
            tc.tile_pool(name="xT", bufs=1) as xtp,
            tc.tile_pool(name="zps", bufs=2, space="PSUM") as zps,
            tc.tile_pool(name="gpool", bufs=2) as gpool,
            tc.tile_pool(name="vpool", bufs=2) as vpool,
            tc.tile_pool(name="opool", bufs=1) as opool,
        ):
            # ---- constants: on the ACT hwdge queue, which is idle at t=0,
            # so the first matmul isn't serialized behind them on SP ----
            wt_sb = consts.tile([128, 256], bf)
            nc.scalar.dma_start(out=wt_sb[:], in_=wt_in[:])
            dc_sb = consts.tile([128, 256], bf)
            nc.scalar.dma_start(out=dc_sb[:], in_=dc_in[:])

            # ---- x load: straight stream on the SP queue; small leading
            # chunks so the matmul pipeline starts as early as possible ----
            xT2 = xtp.tile([128, half], bf)
            edges = [0, 512, 1024, 2048]
            while edges[-1] < half:
                edges.append(min(edges[-1] + 1024, half))
            for c0, c1 in zip(edges[:-1], edges[1:]):
                nc.sync.dma_start(out=xT2[:, c0:c1], in_=xT2_in[:, c0:c1])

            out_sb = opool.tile([128, tiles], f32)

            dbv = dc_sb[:, 0:128].unsqueeze(1).broadcast_to([128, pw, 128])
            bbv = dc_sb[:, 128:256].unsqueeze(1).broadcast_to([128, pw, 128])

            for gi in range(groups):
                # g layout: [128, slot, 256] — slot-major so sigmoid waves are
                # contiguous and level slices are regular strided views.
                g_t = gpool.tile([128, slots * 256], bf)
                gv = g_t[:].rearrange("p (s c) -> p s c", c=256)
                v7 = vpool.tile([128, slots * 128], bf, tag="v7")
                v7v = v7[:].rearrange("p (s c) -> p s c", c=128)
                for hw in range(slots // pw):
                    zt = zps.tile([128, pw * 256], f32)
                    ztv = zt[:].rearrange("p (j c) -> p j c", c=256)
                    for j in range(pw):
                        g = gi * slots + hw * pw + j
                        s, t = divmod(g, tiles // 2)
                        b = 64 * s
                        nc.tensor.matmul(
                            ztv[:, j, 0:NODES],
                            lhsT=xT2[b : b + 34, t * 128 : (t + 1) * 128],
                            rhs=wt_sb[b : b + 34, 0:NODES],
                            start=True,
                            stop=True,
                        )
                    ws = slice(hw * pw, (hw + 1) * pw)
                    nc.scalar.activation(
                        out=gv[:, ws, 0:NODES],
                        in_=ztv[:, :, 0:NODES],
                        func=mybir.ActivationFunctionType.Sigmoid,
                    )
                    # ---- levels 7+6 per sigmoid wave (the bulk of the blend
                    # columns) so blending trails each wave instead of waiting
                    # for the whole group. Waves alternate engines 5:3 to
                    # match DVE/Pool throughput.
                    if gi == groups - 1:
                        # last group: Pool leads, DVE trails (DVE chunks are
                        # faster, shortening the post-sigmoid tail)
                        chunks = [(nc.gpsimd, 0, pw) if hw < 3 else (nc.vector, 0, pw)]
                    else:
                        chunks = [(nc.vector, 0, pw) if (hw % 8) < 5 else (nc.gpsimd, 0, pw)]
                    for eng, w0, w1 in chunks:
                        gvW = gv[:, ws, :][:, w0:w1, :]
                        vW = v7v[:, ws, :][:, w0:w1, :]
                        dbvW = dc_sb[:, 0:128].unsqueeze(1).broadcast_to([128, w1 - w0, 128])
                        bbvW = dc_sb[:, 128:256].unsqueeze(1).broadcast_to([128, w1 - w0, 128])
                        eng.tensor_mul(vW[:, :, 0:128], gvW[:, :, 0:128], dbvW)
                        eng.tensor_add(vW[:, :, 0:128], vW[:, :, 0:128], bbvW)
                        vl6, vr6 = vW[:, :, 0:64], vW[:, :, 64:128]
                        eng.tensor_sub(vl6, vl6, vr6)
                        eng.tensor_mul(vl6, gvW[:, :, 128:192], vl6)
                        eng.tensor_add(vl6, vl6, vr6)

                # ---- levels 5..0 (189 cols/tile), split by slot range so DVE
                # and Pool run independent in-place chains. The last group is
                # split into finer sub-chains to shorten the pipeline tail.
                s_dve = (39 * slots) // 64
                spans = [(nc.vector, 0, s_dve), (nc.gpsimd, s_dve, slots)]
                if gi == groups - 1:
                    spans = [
                        (nc.gpsimd, 0, 16),
                        (nc.gpsimd, 16, 32),
                        (nc.vector, 32, 48),
                        (nc.vector, 48, slots),
                    ]
                for eng, s0, s1 in spans:
                    gvE = gv[:, s0:s1, :]
                    vE = v7v[:, s0:s1, :]
                    for k in range(5, -1, -1):
                        m = 2 ** k
                        off = LEVEL_OFF[k]
                        vl = vE[:, :, 0:m]
                        vr = vE[:, :, m : 2 * m]
                        gk = gvE[:, :, off : off + m]
                        eng.tensor_sub(vl, vl, vr)
                        if k > 0:
                            eng.tensor_mul(vl, gk, vl)
                            eng.tensor_add(vl, vl, vr)
                        else:
                            vo = out_sb[:, gi * slots + s0 : gi * slots + s1]
                            vov = vo.rearrange("p (s c) -> p s c", c=1)
                            eng.tensor_mul(vov, gk, vl)
                            eng.tensor_add(vov, vov, vr)

            for g0 in range(0, groups):
                c0, c1 = g0 * slots, (g0 + 1) * slots
                nc.sync.dma_start(out=out_dram[:, c0:c1], in_=out_sb[:, c0:c1])
    return nc


_CACHE = {}


def _get_nc(rows=ROWS, slots=SLOTS, pw=PW):
    key = (rows, slots, pw)
    if key not in _CACHE:
        nc = build_nc(rows, slots, pw)
        if not nc.is_finalized():
            nc.finalize()
        _CACHE[key] = nc
    return _CACHE[key]


def run_device(x, wt, dc, n_cores=N_CORES, trace=False):
    rows = x.shape[0] // n_cores
    nc = _get_nc(rows)
    in_maps = [
        {
            "xT2": host_xt2(x[i * rows : (i + 1) * rows]),
            "wt": wt,
            "dc": dc,
        }
        for i in range(n_cores)
    ]
    res = run_bass_kernel_spmd(nc, in_maps, list(range(n_cores)), trace=trace)
    out = np.empty((n_cores * rows, 1), np.float32)
    tiles = rows // 128
    for i in range(n_cores):
        dev = res.results[i]["out"].astype(np.float32)  # [128, tiles]
        # tile g covers rows s*rows/2 + 128*t + p with (s, t) = divmod(g, tiles//2)
        core = dev.reshape(128, 2, tiles // 2).transpose(1, 2, 0).reshape(rows)
        out[i * rows : (i + 1) * rows, 0] = core
    return out, res


def kernel(**inputs):
    x = np.asarray(inputs["x"], np.float32).reshape(-1, F)
    wt, dc = host_prep(
        inputs["feature_importances"],
        inputs["feature_splits"],
        inputs["leaf_node_classes"],
    )
    out, _ = run_device(x, wt, dc)
    return out


# revision 35
# speedup vs baseline: 1.0049x; 1.0049x over previous
"""Soft decision-tree forward (nn_DTree) on 8 trn2 NeuronCores.

Strategy (pure data parallel, per the sharding hint):
  - shard x row-wise 8 ways; replicate the tiny tree params.
  - host pre-transposes x into the PE lhsT layout (bf16, bias-ones rows baked
    in), so the device does no transposes at all: the SP queue streams xT2
    straight into SBUF.
  - per core: z = [x|1|1] @ [W | -c_hi | -c_lo] via bf16 PE matmuls into fp32
    PSUM, g = sigmoid(z) on ACT (one instruction per 8-tile PSUM wave), then a
    level-by-level value-tree blend
       V_k = g_k * (V_{k+1,L} - V_{k+1,R}) + V_{k+1,R}
    with nodes pre-permuted (level-major, left-children-first).
  - g is stored slot-major [128, slot, 256] so each sigmoid wave is a single
    strided ACT instruction and every level's gates are regular strided slices.
  - the blend is split between DVE (2x bf16 mode) and Pool so the two engines
    finish together: levels 7+6 are blended per sigmoid wave (8 slots, engines
    alternating 5:3), levels 5..0 once per 64-slot group, in place inside the
    v7 buffer. The level-7 delta/beta constants are step-0 broadcast views of
    a tiny [128, 256] upload, so no replicated tiles are ever materialized.
"""

import numpy as np
import ml_dtypes

import concourse.bass as bass
import concourse.bacc as bacc
import concourse.tile as tile
from concourse import mybir
from concourse.bass_utils import run_bass_kernel_spmd

BF16 = ml_dtypes.bfloat16

F = 32
D = 8
NODES = 255
LEAVES = 256
N_FULL = 262144
N_CORES = 8
ROWS = N_FULL // N_CORES  # 32768 rows per core
SLOTS = 64                # tiles per blend group
PW = 8                    # tiles per PSUM/sigmoid wave

# level-major offsets of each level's gates inside the 255-column block
LEVEL_OFF = {7: 0, 6: 128, 5: 192, 4: 224, 3: 240, 2: 248, 1: 252, 0: 254}


def _orderings():
    """ord[k] = local node order at level k (left-children-first recursion)."""
    ordv = {0: [0]}
    for k in range(7):
        ordv[k + 1] = [2 * i for i in ordv[k]] + [2 * i + 1 for i in ordv[k]]
    col_nodes = []
    for k in range(7, -1, -1):
        base = 2 ** k - 1
        col_nodes += [base + i for i in ordv[k]]
    return ordv, np.array(col_nodes)


def host_prep(feature_importances, feature_splits, leaf_node_classes):
    """Tiny-param preprocessing (O(8K) work): relu/sigmoid/c, node permutation,
    bf16 weight matrix with split bias rows, leaf-blend constants."""
    fi = np.asarray(feature_importances, np.float32).reshape(NODES, F)
    fs = np.asarray(feature_splits, np.float32).reshape(NODES, F)
    cls = np.asarray(leaf_node_classes, np.float32).reshape(LEAVES)

    W = np.maximum(fi, 0.0)
    S = 1.0 / (1.0 + np.exp(-fs))
    c = np.sum(W * S, axis=1)  # (NODES,)

    ordv, col_nodes = _orderings()
    Wp = W[col_nodes]          # (255, 32) permuted level-major
    cp = c[col_nodes]

    c_hi = cp.astype(BF16).astype(np.float32)
    c_lo = (cp - c_hi).astype(np.float32)

    wt = np.zeros((128, 256), BF16)
    for b in (0, 64):  # replicate for both class-half partition groups
        wt[b : b + F, 0:NODES] = Wp.T.astype(BF16)
        wt[b + F, 0:NODES] = (-c_hi).astype(BF16)
        wt[b + F + 1, 0:NODES] = (-c_lo).astype(BF16)

    o7 = np.array(ordv[7])
    delta = (cls[2 * o7] - cls[2 * o7 + 1]).astype(BF16)
    beta = cls[2 * o7 + 1].astype(BF16)
    dc = np.zeros((128, 256), BF16)
    dc[:, 0:128] = delta[None, :]
    dc[:, 128:256] = beta[None, :]
    return wt, dc


def host_xt2(x_core):
    """Build the lhsT layout: [128, ROWS//2] bf16.

    Partitions 0-31: features of rows 0..ROWS/2-1 (class 0)
    Partitions 32-33: ones (bias rows)
    Partitions 64-95: features of rows ROWS/2..ROWS-1 (class 1)
    Partitions 96-97: ones
    """
    half = x_core.shape[0] // 2
    xt = np.empty((128, half), BF16)
    xb = np.ascontiguousarray(x_core.astype(BF16).T)  # (32, ROWS) bf16
    xt[0:32] = xb[:, 0:half]
    xt[64:96] = xb[:, half:]
    xt[32:34] = BF16(1.0)
    xt[96:98] = BF16(1.0)
    xt[34:64] = BF16(0.0)
    xt[98:128] = BF16(0.0)
    return xt


def build_nc(rows=ROWS, slots=SLOTS, pw=PW):
    tiles = rows // 128          # 256
    groups = tiles // slots
    half = rows // 2
    assert tiles % slots == 0 and slots % pw == 0
    bf = mybir.dt.bfloat16
    f32 = mybir.dt.float32

    nc = bacc.Bacc()
    xT2_in = nc.dram_tensor("xT2", [128, half], bf, kind="ExternalInput")
    wt_in = nc.dram_tensor("wt", [128, 256], bf, kind="ExternalInput")
    dc_in = nc.dram_tensor("dc", [128, 256], bf, kind="ExternalInput")
    out_dram = nc.dram_tensor("out", [128, tiles], f32, kind="ExternalOutput")

    with tile.TileContext(nc) as tc:
        with (
            tc.tile_pool(name="consts", bufs=1) as consts,
            tc.tile_pool(name="xT", bufs=1) as xtp,
            tc.tile_pool(name="zps", bufs=2, space="PSUM") as zps,
            tc.tile_pool(name="gpool", bufs=2) as gpool,
            tc.tile_pool(name="vpool", bufs=2) as vpool,
            tc.tile_pool(name="opool", bufs=1) as opool,
        ):
            # ---- constants: on the ACT hwdge queue, which is idle at t=0,
            # so the first matmul isn't serialized behind them on SP ----
            wt_sb = consts.tile([128, 256], bf)
            nc.scalar.dma_start(out=wt_sb[:], in_=wt_in[:])
            dc_sb = consts.tile([128, 256], bf)
            nc.scalar.dma_start(out=dc_sb[:], in_=dc_in[:])

            # ---- x load: straight stream on the SP queue; small leading
            # chunks so the matmul pipeline starts as early as possible ----
            xT2 = xtp.tile([128, half], bf)
            edges = [0, 512, 1024, 2048]
            while edges[-1] < half:
                edges.append(min(edges[-1] + 1024, half))
            for c0, c1 in zip(edges[:-1], edges[1:]):
                nc.sync.dma_start(out=xT2[:, c0:c1], in_=xT2_in[:, c0:c1])

            out_sb = opool.tile([128, tiles], f32)

            dbv = dc_sb[:, 0:128].unsqueeze(1).broadcast_to([128, pw, 128])
            bbv = dc_sb[:, 128:256].unsqueeze(1).broadcast_to([128, pw, 128])

            for gi in range(groups):
                # g layout: [128, slot, 256] — slot-major so sigmoid waves are
                # contiguous and level slices are regular strided views.
                g_t = gpool.tile([128, slots * 256], bf)
                gv = g_t[:].rearrange("p (s c) -> p s c", c=256)
                v7 = vpool.tile([128, slots * 128], bf, tag="v7")
                v7v = v7[:].rearrange("p (s c) -> p s c", c=128)
                for hw in range(slots // pw):
                    zt = zps.tile([128, pw * 256], f32)
                    ztv = zt[:].rearrange("p (j c) -> p j c", c=256)
                    for j in range(pw):
                        g = gi * slots + hw * pw + j
                        s, t = divmod(g, tiles // 2)
                        b = 64 * s
                        nc.tensor.matmul(
                            ztv[:, j, 0:NODES],
                            lhsT=xT2[b : b + 34, t * 128 : (t + 1) * 128],
                            rhs=wt_sb[b : b + 34, 0:NODES],
                            start=True,
                            stop=True,
                        )
                    ws = slice(hw * pw, (hw + 1) * pw)
                    nc.scalar.activation(
                        out=gv[:, ws, 0:NODES],
                        in_=ztv[:, :, 0:NODES],
                        func=mybir.ActivationFunctionType.Sigmoid,
                    )
                    # ---- levels 7+6 per sigmoid wave (the bulk of the blend
                    # columns) so blending trails each wave instead of waiting
                    # for the whole group. Waves alternate engines 5:3 to
                    # match DVE/Pool throughput.
                    if gi == groups - 1:
                        # last group: Pool leads, DVE trails (DVE chunks are
                        # faster, shortening the post-sigmoid tail)
                        chunks = [(nc.gpsimd, 0, pw) if hw < 3 else (nc.vector, 0, pw)]
                    else:
                        chunks = [(nc.vector, 0, pw) if (hw % 8) < 5 else (nc.gpsimd, 0, pw)]
                    for eng, w0, w1 in chunks:
                        gvW = gv[:, ws, :][:, w0:w1, :]
                        vW = v7v[:, ws, :][:, w0:w1, :]
                        dbvW = dc_sb[:, 0:128].unsqueeze(1).broadcast_to([128, w1 - w0, 128])
                        bbvW = dc_sb[:, 128:256].unsqueeze(1).broadcast_to([128, w1 - w0, 128])
                        eng.tensor_mul(vW[:, :, 0:128], gvW[:, :, 0:128], dbvW)
                        eng.tensor_add(vW[:, :, 0:128], vW[:, :, 0:128], bbvW)
                        vl6, vr6 = vW[:, :, 0:64], vW[:, :, 64:128]
                        eng.tensor_sub(vl6, vl6, vr6)
                        eng.tensor_mul(vl6, gvW[:, :, 128:192], vl6)
                        eng.tensor_add(vl6, vl6, vr6)

                # ---- levels 5..0 (189 cols/tile), split by slot range so DVE
                # and Pool run independent in-place chains. The last group is
                # split into finer sub-chains to shorten the pipeline tail.
                s_dve = (37 * slots) // 64
                spans = [(nc.vector, 0, s_dve), (nc.gpsimd, s_dve, slots)]
                if gi == groups - 1:
                    spans = [
                        (nc.gpsimd, 0, 16),
                        (nc.gpsimd, 16, 32),
                        (nc.vector, 32, 48),
                        (nc.vector, 48, slots),
                    ]
                for eng, s0, s1 in spans:
                    gvE = gv[:, s0:s1, :]
                    vE = v7v[:, s0:s1, :]
                    for k in range(5, -1, -1):
                        m = 2 ** k
                        off = LEVEL_OFF[k]
                        vl = vE[:, :, 0:m]
                        vr = vE[:, :, m : 2 * m]
                        gk = gvE[:, :, off : off + m]
                        eng.tensor_sub(vl, vl, vr)
                        if k > 0:
                            eng.tensor_mul(vl, gk, vl)
                            eng.tensor_add(vl, vl, vr)
                        else:
                            vo = out_sb[:, gi * slots + s0 : gi * slots + s1]
                            vov = vo.rearrange("p (s c) -> p s c", c=1)
                            eng.tensor_mul(vov, gk, vl)
                            eng.tensor_add(vov, vov, vr)

            for g0 in range(0, groups):
                c0, c1 = g0 * slots, (g0 + 1) * slots
                nc.sync.dma_start(out=out_dram[:, c0:c1], in_=out_sb[:, c0:c1])
    return nc


_CACHE = {}


def _get_nc(rows=ROWS, slots=SLOTS, pw=PW):
    key = (rows, slots, pw)
    if key not in _CACHE:
        nc = build_nc(rows, slots, pw)
        if not nc.is_finalized():
            nc.finalize()
        _CACHE[key] = nc
    return _CACHE[key]


def run_device(x, wt, dc, n_cores=N_CORES, trace=False):
    rows = x.shape[0] // n_cores
    nc = _get_nc(rows)
    in_maps = [
        {
            "xT2": host_xt2(x[i * rows : (i + 1) * rows]),
            "wt": wt,
            "dc": dc,
        }
        for i in range(n_cores)
    ]
    res = run_bass_kernel_spmd(nc, in_maps, list(range(n_cores)), trace=trace)
    out = np.empty((n_cores * rows, 1), np.float32)
    tiles = rows // 128
    for i in range(n_cores):
        dev = res.results[i]["out"].astype(np.float32)  # [128, tiles]
        # tile g covers rows s*rows/2 + 128*t + p with (s, t) = divmod(g, tiles//2)
        core = dev.reshape(128, 2, tiles // 2).transpose(1, 2, 0).reshape(rows)
        out[i * rows : (i + 1) * rows, 0] = core
    return out, res


def kernel(**inputs):
    x = np.asarray(inputs["x"], np.float32).reshape(-1, F)
    wt, dc = host_prep(
        inputs["feature_importances"],
        inputs["feature_splits"],
        inputs["leaf_node_classes"],
    )
    out, _ = run_device(x, wt, dc)
    return out
